# revision 7
# baseline (speedup 1.0000x reference)
"""CutMix kernel for 8 Trainium2 NeuronCores.

Strategy (pure data-parallel, host-side gather for the permutation):
  - Host computes per-sample box geometry (cut/start/end) exactly as the
    reference does (all f32 ops bit-match jax on CPU).
  - Core c gets samples [16c, 16c+16) plus the partner samples
    x[perm[...]] gathered on host, so the device never needs cross-core
    communication.
  - One SPMD Bass program is built per call with every core's geometry
    baked in as static DMA access patterns; each DMA is predicated on
    partition_id == c, so only the owning core executes it.
  - Per sample, the output image is assembled by 5 disjoint DRAM->DRAM
    band copies (top/bottom/left/right from the local sample, the box
    from the partner sample). Label mixing runs on the vector engine.

Note: gpsimd (SWDGE) cond-DMAs crash the exec unit on HW; only the two
HWDGE engines (sync/SP and scalar/ACT) issue the predicated DMAs.
"""

import numpy as np

import concourse.bass as bass
import concourse.tile as tile
from concourse import bacc, mybir
from concourse.bass_utils import run_bass_kernel_spmd

B, C, H, W = 128, 3, 384, 384
NCLS = 1000
M = 8          # cores
BS = B // M    # samples per core


def _geometry(lam, siv):
    # mirror reference.py bit-for-bit in f32
    lam = np.asarray(lam, np.float32)
    siv = np.asarray(siv, np.float32)
    cut = (np.float32(H) * np.sqrt(np.float32(1.0) - lam)).astype(np.int32)
    start = (siv * (H - cut).astype(np.float32)).astype(np.int32)
    end = start + cut
    return cut, start, end


def _build_program(start, end, engines_mode="sync+scalar", repeat=1):
    nc = bacc.Bacc("TRN2", target_bir_lowering=False, debug=False, num_devices=M)
    f32 = mybir.dt.float32

    x_loc = nc.dram_tensor("x_loc", [BS, C, H, W], f32, kind="ExternalInput")
    x_mix = nc.dram_tensor("x_mix", [BS, C, H, W], f32, kind="ExternalInput")
    lab_loc = nc.dram_tensor("lab_loc", [BS, NCLS], f32, kind="ExternalInput")
    lab_mix = nc.dram_tensor("lab_mix", [BS, NCLS], f32, kind="ExternalInput")
    lam_t = nc.dram_tensor("lam", [BS, 1], f32, kind="ExternalInput")
    x2 = nc.dram_tensor("x2", [BS, C, H, W], f32, kind="ExternalOutput")
    labout = nc.dram_tensor("labout", [BS, NCLS], f32, kind="ExternalOutput")

    with tile.TileContext(nc) as tc:
        # mixed_labels = lam*a + (1-lam)*b computed as b + lam*(a-b)
        with tc.tile_pool(name="lab", bufs=1) as pool:
            a = pool.tile([BS, NCLS], f32)
            nc.sync.dma_start(a[:], lab_loc[:])
            bt = pool.tile([BS, NCLS], f32)
            nc.sync.dma_start(bt[:], lab_mix[:])
            lt = pool.tile([BS, 1], f32)
            nc.sync.dma_start(lt[:], lam_t[:])
            d = pool.tile([BS, NCLS], f32)
            nc.vector.tensor_sub(out=d[:], in0=a[:], in1=bt[:])
            nc.vector.tensor_scalar_mul(d[:], d[:], lt[:])
            nc.vector.tensor_add(out=d[:], in0=d[:], in1=bt[:])
            nc.sync.dma_start(labout[:], d[:])

        engines = [
            {"sync": nc.sync, "scalar": nc.scalar, "gpsimd": nc.gpsimd}[name]
            for name in engines_mode.split("+")
        ]
        pids = [eng.partition_id() for eng in engines]
        conds = [[pid == c for c in range(M)] for pid in pids]

        k = 0

        def emit(dst, src, c):
            nonlocal k
            i = k % len(engines)
            k += 1
            # width-1 strips opt() away their contiguous [1,1] inner dim;
            # allow the resulting degenerate AP (tiny byte counts)
            with nc.allow_non_contiguous_dma(reason="narrow cutmix strip"):
                engines[i].dma_start(dst, src, cond=conds[i][c])

        for _ in range(repeat):
            for j in range(BS):
                for c in range(M):
                    s = int(start[c * BS + j])
                    e = int(end[c * BS + j])
                    if s > 0:
                        emit(x2[j, :, 0:s, :], x_loc[j, :, 0:s, :], c)
                    if e < H:
                        emit(x2[j, :, e:H, :], x_loc[j, :, e:H, :], c)
                    if e > s:
                        if s > 0:
                            emit(x2[j, :, s:e, 0:s], x_loc[j, :, s:e, 0:s], c)
                        if e < W:
                            emit(x2[j, :, s:e, e:W], x_loc[j, :, s:e, e:W], c)
                        emit(x2[j, :, s:e, s:e], x_mix[j, :, s:e, s:e], c)

    nc.compile()
    return nc


def _shard_inputs(x, labels, lam, perm):
    in_maps = []
    for c in range(M):
        idx = slice(c * BS, (c + 1) * BS)
        pidx = perm[idx]
        in_maps.append(
            {
                "x_loc": x[idx],
                "x_mix": x[pidx],
                "lab_loc": labels[idx],
                "lab_mix": labels[pidx],
                "lam": lam[idx].reshape(BS, 1),
            }
        )
    return in_maps


def kernel(
    x,
    labels,
    lambdaVal,
    start_idx_vals,
    mix_candidate_idxs,
    _engines_mode="sync+scalar",
):
    x = np.ascontiguousarray(np.asarray(x, np.float32))
    labels = np.ascontiguousarray(np.asarray(labels, np.float32))
    lam = np.asarray(lambdaVal, np.float32)
    siv = np.asarray(start_idx_vals, np.float32)
    perm = np.asarray(mix_candidate_idxs, np.int32)

    cut, start, end = _geometry(lam, siv)

    nc = _build_program(start, end, engines_mode=_engines_mode)

    in_maps = _shard_inputs(x, labels, lam, perm)

    res = run_bass_kernel_spmd(nc, in_maps, core_ids=list(range(M)))

    x2 = np.concatenate([res.results[c]["x2"] for c in range(M)], axis=0)
    mixed = np.concatenate([res.results[c]["labout"] for c in range(M)], axis=0)
    return x2, mixed
